# revision 19
# baseline (speedup 1.0000x reference)
"""Trainium2 Bass kernel for nn_AttentionBlock (GroupNorm + MHSA + proj + residual).

Sharding: pure data-parallel over batch. B=8 == 8 cores, one batch element per
core, zero collectives. Each core computes:
    xf = x[b].reshape(C, T)
    xn = GroupNorm32(xf) * gn_w + gn_b
    qkv = w_qkv @ xn + b_qkv            (channel order permuted to [Q|K|V] blocks)
    per head: S^T[s,t] = (q^T k)/8 ; E = exp(S^T) ; a' = [v^T|1]^T E  (denominator
              via ones column) ; a = a'/d
    y = xf + w_proj @ a + b_eff         (b_eff = w_proj @ b_v + b_proj, host-folded)

Matmuls run as float32r (full PE rate at N>=512). Weights are transposed and
head-deinterleaved on the host so no on-device transposes are needed anywhere:
V is produced directly in [s, c] layout by computing that part of the QKV matmul
in the flipped orientation (lhsT = xn).
"""

import sys
import numpy as np

for _p in ("/opt/trn_rl_repo", "/opt/pypackages"):
    if _p not in sys.path:
        sys.path.append(_p)

import concourse.bass as bass
import concourse.bacc as bacc
import concourse.tile as tile
from concourse import mybir
from concourse._compat import with_exitstack

F32 = mybir.dt.float32
F32R = mybir.dt.float32r
AF = mybir.ActivationFunctionType
OP = mybir.AluOpType

B, C, T = 8, 512, 1024
HEADS, GROUPS, CHD = 8, 32, 64  # heads, groups, head dim
NCORES = 8
EPS = 1e-5

# Module-level knobs/results for test harness use (harness calls kernel() only).
TRACE = False
LAST_RESULTS = None
_NC_CACHE = None


@with_exitstack
def _body(ctx, tc, aps):
    nc = tc.nc
    ctx.enter_context(nc.allow_low_precision(
        reason="float32r tiles: rounded fp32 matmul operands, fp32 accumulate"))
    x_in, wq_in, wp_in = aps["x_in"], aps["wq_in"], aps["wp_in"]
    y_out = aps["y_out"]

    persist = ctx.enter_context(tc.tile_pool(name="persist", bufs=1))
    scr = ctx.enter_context(tc.tile_pool(name="scr", bufs=2))
    outp = ctx.enter_context(tc.tile_pool(name="outp", bufs=2))
    psum = ctx.enter_context(tc.tile_pool(name="psum", bufs=6, space="PSUM"))
    pstat = ctx.enter_context(tc.tile_pool(name="pstat", bufs=1, space="PSUM"))

    # ---- persistent SBUF tiles ----
    X = persist.tile([128, 4, T], F32, name="X")      # input, also residual
    Qb = persist.tile([128, 4, T], F32R, name="Qb")    # q channels (head-pair per chunk)
    Kb = persist.tile([128, 4, T], F32R, name="Kb")    # k channels
    # v^T: [s%128, s//128, head*66 + (0:64 v | col 64 ones | pad)]; the ones
    # column makes each AV matmul also emit the softmax denominator on psum
    # partition 64.
    VT = persist.tile([128, 8, 8 * 66], F32R, name="VT")
    Ab = persist.tile([128, 4, T], F32R, name="Ab")    # attention output (pre-proj)
    WP = persist.tile([128, 4, C], F32R, name="WP")    # w_proj^T
    dsb = persist.tile([HEADS, T], F32, name="dsb")   # softmax denominators
    rdsb = persist.tile([HEADS, T], F32R, name="rdsb")
    mq = persist.tile([128, 4, 2], F32, name="mq")    # per-(partition,chunk) [mean, E[x^2]]
    sst = persist.tile([8, 8], F32, name="sst")       # per-group stats
    Asc = persist.tile([128, 4], F32, name="Asc")     # GN scale per channel
    Bsc = persist.tile([128, 4], F32, name="Bsc")     # GN shift per channel

    smalls = {}
    for nm in ("bq", "bk", "be", "gw", "gb"):
        smalls[nm] = persist.tile([128, 4], F32, name=nm + "_t")
        nc.sync.dma_start(smalls[nm], aps[nm + "_in"].ap())
    ind_t = persist.tile([128, 8], F32, name="ind_t")
    nc.sync.dma_start(ind_t, aps["ind_in"].ap())
    ind2_t = persist.tile([8, 128], F32, name="ind2_t")
    nc.sync.dma_start(ind2_t, aps["ind2_in"].ap())
    indh_t = persist.tile([8, C], F32R, name="indh_t")
    nc.sync.dma_start(indh_t, aps["indh_in"].ap())
    ones_t = persist.tile([128, 64], F32R, name="ones_t")
    nc.sync.dma_start(ones_t, aps["ones_in"].ap())
    epsb = persist.tile([8, 1], F32, name="epsb")
    nc.vector.memset(epsb, EPS)

    xr = x_in.ap().rearrange("(j p) t -> p j t", p=128)
    for j in range(4):
        nc.sync.dma_start(X[:, j], xr[:, j])
    wpr = wp_in.ap().rearrange("(j p) m -> p j m", p=128)
    for j in range(4):
        nc.sync.dma_start(WP[:, j], wpr[:, j])

    # ---- phase 1+2: GroupNorm and QKV (wpool tiles die after this) ----
    with tc.tile_pool(name="wpool", bufs=1) as wpool:
        WQ = wpool.tile([128, 4, 3 * C], F32R, name="WQ")
        XN = wpool.tile([128, 4, T], F32R, name="XN")
        wqr = wq_in.ap().rearrange("(j p) m -> p j m", p=128)
        for j in range(4):
            nc.sync.dma_start(WQ[:, j], wqr[:, j])

        # GroupNorm stats: bn_stats over free dim per partition, then aggregate
        # groups of 16 partitions with a tiny indicator matmul (values 1/16).
        for j in range(4):
            st6 = scr.tile([128, 2, 6], F32, name="st6", tag="st6")
            nc.vector.bn_stats(st6[:, 0], X[:, j, 0:512])
            nc.vector.bn_stats(st6[:, 1], X[:, j, 512:1024])
            nc.vector.bn_aggr(mq[:, j], st6)
        tm = scr.tile([128, 4], F32, name="tm", tag="tm")
        nc.vector.tensor_mul(tm, mq[:, :, 0], mq[:, :, 0])
        nc.vector.tensor_add(mq[:, :, 1], mq[:, :, 1], tm)  # E[x^2] per partition

        pst = pstat.tile([8, 8], F32, name="pst")
        nc.tensor.matmul(pst, lhsT=ind_t, rhs=mq.rearrange("p a b -> p (a b)"),
                         start=True, stop=True)
        nc.vector.tensor_copy(sst, pst)
        sstv = sst.rearrange("p (a b) -> p a b", b=2)  # [8, 4, 2] (mean, E[x^2])
        tms = scr.tile([8, 4], F32, name="tms", tag="tms")
        nc.vector.tensor_mul(tms, sstv[:, :, 0], sstv[:, :, 0])
        nc.vector.tensor_tensor(sstv[:, :, 1], sstv[:, :, 1], tms, op=OP.subtract)
        nc.scalar.activation(sstv[:, :, 1], sstv[:, :, 1], AF.Sqrt, bias=epsb)
        nc.vector.reciprocal(sstv[:, :, 1], sstv[:, :, 1])  # rstd per group

        pch = pstat.tile([128, 8], F32, name="pch")
        nc.tensor.matmul(pch, lhsT=ind2_t, rhs=sst, start=True, stop=True)
        pchv = pch.rearrange("p (a b) -> p a b", b=2)
        nc.vector.tensor_mul(Asc, pchv[:, :, 1], smalls["gw"])
        nc.vector.tensor_mul(Bsc, pchv[:, :, 0], Asc)
        nc.vector.tensor_tensor(Bsc, smalls["gb"], Bsc, op=OP.subtract)
        for j in range(4):
            nc.vector.tensor_scalar(XN[:, j], X[:, j], Asc[:, j:j + 1],
                                    Bsc[:, j:j + 1], op0=OP.mult, op1=OP.add)

        # QKV: Q and K in [o, t] orientation
        for m in range(8):
            dst = Qb if m < 4 else Kb
            bias = smalls["bq"] if m < 4 else smalls["bk"]
            mj = m % 4
            for th in range(2):
                ps = psum.tile([128, 512], F32, name="ps", tag="ps")
                for kc in range(4):
                    nc.tensor.matmul(ps,
                                     lhsT=WQ[:, kc, m * 128:(m + 1) * 128],
                                     rhs=XN[:, kc, th * 512:(th + 1) * 512],
                                     start=(kc == 0), stop=(kc == 3))
                nc.vector.tensor_scalar_add(dst[:, mj, th * 512:(th + 1) * 512],
                                            ps, bias[:, mj:mj + 1])
        # V in flipped [t, c_v] orientation -> lands directly as v^T
        # bake the per-head ones column (col 64 of each 66-wide block)
        nc.vector.tensor_copy(
            VT.rearrange("p a (h w) -> p a h w", w=66)[:, :, :, 64:65],
            ones_t.rearrange("p (a h w) -> p a h w", a=8, h=8, w=1))
        for mt in range(8):
            ps = psum.tile([128, 512], F32, name="ps", tag="ps")
            for kc in range(4):
                nc.tensor.matmul(ps,
                                 lhsT=XN[:, kc, mt * 128:(mt + 1) * 128],
                                 rhs=WQ[:, kc, 2 * C:3 * C],
                                 start=(kc == 0), stop=(kc == 3))
            vtv = VT[:, mt].rearrange("p (h w) -> p h w", w=66)
            nc.vector.tensor_copy(vtv[:, :, 0:64],
                                  ps.rearrange("p (h w) -> p h w", w=64))

    # ---- phase 3: per-head attention ----
    with tc.tile_pool(name="epool", bufs=2) as epool:
        for h in range(HEADS):
            j, slot = divmod(h, 2)
            pb = 64 * slot
            E = epool.tile([128, 8, T], F32R, name="E", tag="E")
            for st in range(8):
                for th in range(2):
                    ps = psum.tile([128, 512], F32, name="ps", tag="ps")
                    nc.tensor.matmul(ps,
                                     lhsT=Kb[pb:pb + 64, j, st * 128:(st + 1) * 128],
                                     rhs=Qb[pb:pb + 64, j, th * 512:(th + 1) * 512],
                                     start=True, stop=True)
                    # scores scale 1/8 folded into exp's input scale
                    nc.scalar.activation(E[:, st, th * 512:(th + 1) * 512], ps,
                                         AF.Exp, scale=0.125)
            for th in range(2):
                pa = psum.tile([128, 512], F32, name="ps", tag="ps")
                for st in range(8):
                    nc.tensor.matmul(pa[0:65],
                                     lhsT=VT[:, st, 66 * h:66 * h + 65],
                                     rhs=E[:, st, th * 512:(th + 1) * 512],
                                     start=(st == 0), stop=(st == 7))
                nc.vector.tensor_copy(Ab[pb:pb + 64, j, th * 512:(th + 1) * 512],
                                      pa[0:64])
                # denominator row: psum partition 64 -> sbuf stage (same
                # partition), then cross-partition DMA into dsb row h
                stage = scr.tile([65, 512], F32, name="stage", tag="dstage")
                nc.vector.tensor_copy(stage[64:65], pa[64:65])
                nc.sync.dma_start(dsb[h:h + 1, th * 512:(th + 1) * 512],
                                  stage[64:65])

        # normalize: a /= d, d broadcast over each head's 64 channels via matmul
        nc.vector.reciprocal(rdsb, dsb)
        for jj in range(4):
            for th in range(2):
                bc = psum.tile([128, 512], F32, name="ps", tag="ps")
                nc.tensor.matmul(bc,
                                 lhsT=indh_t[:, jj * 128:(jj + 1) * 128],
                                 rhs=rdsb[:, th * 512:(th + 1) * 512],
                                 start=True, stop=True)
                nc.vector.tensor_mul(Ab[:, jj, th * 512:(th + 1) * 512],
                                     Ab[:, jj, th * 512:(th + 1) * 512], bc)

    # ---- phase 4: proj + bias + residual ----
    yr = y_out.ap().rearrange("(m p) t -> p m t", p=128)
    for mo in range(4):
        ot = outp.tile([128, T], F32, name="OT", tag="OT")
        for th in range(2):
            pp = psum.tile([128, 512], F32, name="ps", tag="ps")
            for kc in range(4):
                nc.tensor.matmul(pp,
                                 lhsT=WP[:, kc, mo * 128:(mo + 1) * 128],
                                 rhs=Ab[:, kc, th * 512:(th + 1) * 512],
                                 start=(kc == 0), stop=(kc == 3))
            nc.vector.tensor_scalar_add(ot[:, th * 512:(th + 1) * 512], pp,
                                        smalls["be"][:, mo:mo + 1])
        nc.vector.tensor_add(ot, ot, X[:, mo])
        nc.sync.dma_start(yr[:, mo], ot)


def build_nc():
    nc = bacc.Bacc("TRN2", target_bir_lowering=False, debug=False)
    aps = {}
    aps["x_in"] = nc.declare_dram_parameter("x_in", [C, T], F32, isOutput=False)
    aps["wq_in"] = nc.declare_dram_parameter("wq_in", [C, 3 * C], F32R, isOutput=False)
    aps["wp_in"] = nc.declare_dram_parameter("wp_in", [C, C], F32R, isOutput=False)
    for nm in ("bq_in", "bk_in", "be_in", "gw_in", "gb_in"):
        aps[nm] = nc.declare_dram_parameter(nm, [128, 4], F32, isOutput=False)
    aps["ind_in"] = nc.declare_dram_parameter("ind_in", [128, 8], F32, isOutput=False)
    aps["ind2_in"] = nc.declare_dram_parameter("ind2_in", [8, 128], F32, isOutput=False)
    aps["indh_in"] = nc.declare_dram_parameter("indh_in", [8, C], F32R, isOutput=False)
    aps["ones_in"] = nc.declare_dram_parameter("ones_in", [128, 64], F32R, isOutput=False)
    aps["y_out"] = nc.declare_dram_parameter("y_out", [C, T], F32, isOutput=True)
    with tile.TileContext(nc) as tc:
        _body(tc, aps)
    nc.compile()
    return nc


def host_inputs(x, gn_w, gn_b, w_qkv, b_qkv, w_proj, b_proj):
    """Host-side prep: shard x over batch; transpose + head-deinterleave weights."""
    x = np.asarray(x, np.float32)
    gn_w = np.asarray(gn_w, np.float32)
    gn_b = np.asarray(gn_b, np.float32)
    w_qkv = np.asarray(w_qkv, np.float32)
    b_qkv = np.asarray(b_qkv, np.float32)
    w_proj = np.asarray(w_proj, np.float32)
    b_proj = np.asarray(b_proj, np.float32)

    ch = C // HEADS
    # legacy improved-diffusion split: qkv reshaped to [H, 3*ch, T] before q/k/v
    # split, so head h's q rows are 192h..192h+64, k: +64.., v: +128..
    perm = np.empty(3 * C, np.int64)
    for h in range(HEADS):
        base = 3 * ch * h
        perm[ch * h: ch * (h + 1)] = np.arange(base, base + ch)
        perm[C + ch * h: C + ch * (h + 1)] = np.arange(base + ch, base + 2 * ch)
        perm[2 * C + ch * h: 2 * C + ch * (h + 1)] = np.arange(base + 2 * ch, base + 3 * ch)

    wqT = np.ascontiguousarray(w_qkv[perm].T)          # [C, 3C]
    wpT = np.ascontiguousarray(w_proj.T)               # [C, C]
    bp = b_qkv[perm]
    bq = np.ascontiguousarray(bp[0:C].reshape(4, 128).T)       # [128, 4]
    bk = np.ascontiguousarray(bp[C:2 * C].reshape(4, 128).T)
    b_eff = w_proj @ bp[2 * C:3 * C] + b_proj          # fold v-bias through proj
    be = np.ascontiguousarray(b_eff.reshape(4, 128).T.astype(np.float32))
    gw = np.ascontiguousarray(gn_w.reshape(4, 128).T)
    gb = np.ascontiguousarray(gn_b.reshape(4, 128).T)

    p = np.arange(128)
    ind = (p[:, None] // 16 == np.arange(8)[None, :]).astype(np.float32) / 16.0
    ind2 = np.ascontiguousarray(ind.T * 16.0)          # [8, 128] of 1.0
    cc = np.arange(C)
    indh = (np.arange(8)[:, None] == cc[None, :] // 64).astype(np.float32)  # [8, C]

    shared = dict(wq_in=wqT, wp_in=wpT, bq_in=bq, bk_in=bk, be_in=be,
                  gw_in=gw, gb_in=gb, ind_in=ind, ind2_in=ind2, indh_in=indh,
                  ones_in=np.ones((128, 64), np.float32))
    in_maps = []
    for b in range(B):
        m = dict(shared)
        m["x_in"] = np.ascontiguousarray(x[b].reshape(C, T))
        in_maps.append(m)
    return in_maps


def kernel(x, dummy, gn_w, gn_b, w_qkv, b_qkv, w_proj, b_proj):
    global _NC_CACHE, LAST_RESULTS
    from concourse.bass_utils import run_bass_kernel_spmd

    if _NC_CACHE is None:
        _NC_CACHE = build_nc()
    nc = _NC_CACHE
    in_maps = host_inputs(x, gn_w, gn_b, w_qkv, b_qkv, w_proj, b_proj)
    res = run_bass_kernel_spmd(nc, in_maps, core_ids=list(range(NCORES)),
                               trace=TRACE)
    LAST_RESULTS = res
    y = np.stack([res.results[b]["y_out"] for b in range(B)])
    return y.reshape(B, C, 32, 32).astype(np.float32)


if __name__ == "__main__":
    # smoke test with random data
    rng = np.random.default_rng(0)
    out = kernel(
        rng.standard_normal((B, C, 32, 32), np.float32), 0,
        np.ones(C, np.float32), np.zeros(C, np.float32),
        rng.standard_normal((3 * C, C), np.float32) * C ** -0.5,
        np.zeros(3 * C, np.float32),
        rng.standard_normal((C, C), np.float32) * C ** -0.5,
        np.zeros(C, np.float32),
    )
    print(out.shape, out.dtype)
